# revision 5
# baseline (speedup 1.0000x reference)
"""NT-Xent (SimCLR) contrastive loss on 8 Trainium2 NeuronCores.

Strategy — symmetric wrapped-band decomposition, uniform SPMD:
  zhat = z/||z|| (host; fp8-e4m3 quantized once). sim = 2*(zhat @ zhat.T)
  is symmetric, so each of the 64 row-chunks (128 rows) only needs the
  half-band of column-chunks t = 0..32 to its right (mod 64): pairs at
  chunk distance 1..31 are computed from the lower-numbered row, distance
  32 from the row-chunk < 32. Row sums of exp come from the ACT
  accumulator; the mirrored contributions (row sums of the other member
  of each pair) come from PE column-sum matmuls over the fp8 exp tiles.

  Core c owns chunks {c, c+8, .., c+56} and receives zhat columns
  pre-rotated by 128c (host), so the instruction stream is identical on
  every core: chunk a's band is always columns [1024a, 1024a+4224) of
  the rotated, circularly-extended operand. Cores output a [128, 64]
  partial row-sum grid (rotated chunk order) + their rows' positive-pair
  dots; the host un-rotates, adds the 8 partials, and applies
  ln(rowsum - e^2) and the mean.
"""

import os
import sys

for _p in ("/opt/trn_rl_repo", os.path.expanduser("~/.axon_site/_ro/trn_rl_repo")):
    if os.path.isdir(_p) and _p not in sys.path:
        sys.path.insert(0, _p)

import ml_dtypes
import numpy as np

import concourse.bass as bass
import concourse.tile as tile
from concourse import bacc, mybir
from concourse.bass_utils import run_bass_kernel_spmd

_dt = mybir.dt
_AF = mybir.ActivationFunctionType
_DR = mybir.MatmulPerfMode.DoubleRow

B = 4096
D = 256
N = 2 * B
NCORE = 8
P = 128
NBLK = N // P              # 64 chunks of 128 rows
WIN = 1536                 # sim/exp window width (3 PSUM banks)
EXT = 1024 * 7 + 32 * P    # 11264: rotated operand incl. circular margin
E2 = float(np.exp(np.float32(2.0)))

# chunk a (rotated row-chunk 8a) spans band blocks t in [0, TA[a]]
TA = [32, 32, 32, 32, 31, 31, 31, 31]
DMA_PIECES = [(0, 4224), (4224, 8320), (8320, EXT)]


def _windows(a):
    """(start, width) windows of chunk a's band within the rotated operand."""
    s0 = 1024 * a
    end = s0 + (TA[a] + 1) * P
    out = []
    s = s0
    while s < end:
        w = min(WIN, end - s)
        out.append((s, w))
        s += w
    return out


def _covering():
    """cs column u -> ordered [(a, t), ...] contributing colsum matmuls."""
    cov = {u: [] for u in range(NBLK)}
    for a in range(8):
        for t in range(1, TA[a] + 1):
            cov[(8 * a + t) % NBLK].append((a, t))
    return cov


def _program(nc, tc, zh, zq, zp, out_part, out_pos):
    f32, fp8 = _dt.float32, _dt.float8e4
    X = mybir.AxisListType.X
    from contextlib import ExitStack

    ctx = ExitStack()
    with ctx:
        const = ctx.enter_context(tc.tile_pool(name="const", bufs=1))
        persist = ctx.enter_context(tc.tile_pool(name="persist", bufs=1))
        epool = ctx.enter_context(tc.tile_pool(name="epool", bufs=3))
        simps = ctx.enter_context(tc.tile_pool(name="simps", bufs=2, space="PSUM"))
        csps = ctx.enter_context(tc.tile_pool(name="csps", bufs=2, space="PSUM"))

        ones8 = const.tile([P, 1], fp8, tag="ones8")
        nc.vector.memset(ones8[:], 1.0)
        # warm the Exp table while DMAs run
        warm_in = const.tile([P, 1], f32, tag="warm_in")
        nc.vector.memset(warm_in[:], 0.0)
        warm_out = const.tile([P, 1], f32, tag="warm_out")
        nc.scalar.activation(warm_out[:], warm_in[:], _AF.Exp)

        # ---- rotated zhat fp8 operand, loaded in 3 pieces
        zt = persist.tile([P, 2, EXT], fp8, tag="zt")
        for lo, hi in DMA_PIECES:
            nc.sync.dma_start(
                zt[:, :, lo:hi],
                zh[:, lo:hi].rearrange("(c p) j -> p c j", p=P),
            )

        # ---- positive pairs: pos[p, a] = 2 * sum_d zq[p,a,d]*zp[p,a,d]
        q_sb = persist.tile([P, 8 * D], f32, tag="q_sb")
        nc.gpsimd.dma_start(q_sb[:], zq[:, :].rearrange("(a p) d -> p a d", p=P))
        p_sb = persist.tile([P, 8 * D], f32, tag="p_sb")
        nc.gpsimd.dma_start(p_sb[:], zp[:, :].rearrange("(a p) d -> p a d", p=P))
        ddm = persist.tile([P, 8 * D], f32, tag="ddm")
        nc.vector.tensor_mul(ddm[:], q_sb[:], p_sb[:])
        dd8 = persist.tile([P, 8], f32, tag="dd8")
        nc.vector.reduce_sum(
            dd8[:], ddm[:].rearrange("p (a d) -> p a d", d=D), axis=X
        )
        pos8 = persist.tile([P, 8], f32, tag="pos8")
        nc.vector.tensor_scalar_mul(pos8[:], dd8[:], 2.0)
        nc.sync.dma_start(out_pos[:, :], pos8[:])

        # ---- main loop
        sacc = persist.tile([P, 24], f32, tag="sacc")
        out_sb = persist.tile([P, NBLK], f32, tag="out_sb")
        nc.vector.memset(out_sb[:], 0.0)

        pending = None  # (a, jb0, nblocks, Etile)

        def emit_colsum(p):
            # one PSUM bank per window; a single start -> accumulate group
            # (start=True zeroes the whole 2KB bank, so exactly one per tile)
            a, jb0, nb, et = p
            blocks = [j for j in range(nb) if jb0 + j != 8 * a]
            cst = csps.tile([P, 512], f32, tag="cs")
            for i, j in enumerate(blocks):
                nc.tensor.matmul(
                    cst[:, j : j + 1],
                    et[:, P * j : P * (j + 1)],
                    ones8[:],
                    start=(i == 0),
                    stop=(i == len(blocks) - 1),
                    skip_group_check=True,
                )
            # drain into out_sb at u = (jb0+j) % 64, splitting at the wrap
            j0 = blocks[0]
            runs = []
            while j0 <= blocks[-1]:
                u0 = (jb0 + j0) % NBLK
                ln = min(blocks[-1] - j0 + 1, NBLK - u0)
                runs.append((j0, u0, ln))
                j0 += ln
            for j, u, ln in runs:
                nc.vector.tensor_add(
                    out_sb[:, u : u + ln],
                    out_sb[:, u : u + ln],
                    cst[:, j : j + ln],
                )

        slot = 0
        for a in range(8):
            lhs0 = 1024 * a
            for start, w in _windows(a):
                ps = simps.tile([P, WIN], f32, tag="sim")
                for bank in range(0, w, 512):
                    bend = min(bank + 512, w)
                    for col in range(bank, bend, 256):
                        n = min(256, bend - col)
                        nc.tensor.matmul(
                            ps[:, col : col + n],
                            zt[:, :, lhs0 : lhs0 + P],
                            zt[:, :, start + col : start + col + n],
                            start=(col == bank),
                            stop=(col + n == bend),
                            skip_group_check=True,
                            perf_mode=_DR,
                        )
                et = epool.tile([P, WIN], fp8, tag="E")
                nc.scalar.activation(
                    et[:, :w],
                    ps[:, :w],
                    _AF.Exp,
                    scale=2.0,
                    accum_out=sacc[:, slot : slot + 1],
                )
                slot += 1
                if pending is not None:
                    emit_colsum(pending)
                pending = (a, start // P, w // P, et)
        emit_colsum(pending)

        # ---- add own-row band sums (rotated chunk order)
        rs = persist.tile([P, 8], f32, tag="rs")
        for a in range(8):
            nc.vector.reduce_sum(
                rs[:, a : a + 1],
                sacc[:, 3 * a : 3 * a + 3].rearrange("p (o w) -> p o w", o=1),
                axis=X,
            )
            u = 8 * a
            nc.vector.tensor_add(
                out_sb[:, u : u + 1], out_sb[:, u : u + 1], rs[:, a : a + 1]
            )
        nc.sync.dma_start(out_part[:, :], out_sb[:])


_NC_CACHE = None


def _build():
    global _NC_CACHE
    if _NC_CACHE is not None:
        return _NC_CACHE
    nc = bacc.Bacc("TRN2", target_bir_lowering=False, debug=False, num_devices=NCORE)
    zh = nc.dram_tensor("zh", [D, EXT], _dt.float8e4, kind="ExternalInput")
    zq = nc.dram_tensor("zq", [P * 8, D], _dt.float32, kind="ExternalInput")
    zp = nc.dram_tensor("zp", [P * 8, D], _dt.float32, kind="ExternalInput")
    out_part = nc.dram_tensor("partial", [P, NBLK], _dt.float32, kind="ExternalOutput")
    out_pos = nc.dram_tensor("pos", [P, 8], _dt.float32, kind="ExternalOutput")
    with tile.TileContext(nc) as tc:
        _program(nc, tc, zh, zq, zp, out_part, out_pos)
    nc.compile()
    _NC_CACHE = nc
    return nc


def make_in_maps(z_i, z_j):
    z = np.concatenate([z_i, z_j], axis=0).astype(np.float32)
    zhat = z / np.linalg.norm(z, axis=1, keepdims=True)
    zh8 = np.ascontiguousarray(zhat.T).astype(ml_dtypes.float8_e4m3)  # [D, N]
    in_maps = []
    for c in range(NCORE):
        rot = np.roll(zh8, -P * c, axis=1)
        ext = np.concatenate([rot, rot[:, : EXT - N]], axis=1)
        own = [(c + 8 * a) % NBLK for a in range(8)]
        zq = np.concatenate([zhat[P * m : P * (m + 1)] for m in own], axis=0)
        zp = np.concatenate(
            [zhat[P * ((m + 32) % NBLK) : P * ((m + 32) % NBLK) + P] for m in own],
            axis=0,
        )
        in_maps.append(
            {
                "zh": np.ascontiguousarray(ext),
                "zq": np.ascontiguousarray(zq),
                "zp": np.ascontiguousarray(zp),
            }
        )
    return in_maps


def kernel(z_i, z_j, **kw):
    nc = _build()
    in_maps = make_in_maps(z_i, z_j)
    res = run_bass_kernel_spmd(nc, in_maps, core_ids=list(range(NCORE)), **kw)
    total = np.zeros((P, NBLK), dtype=np.float64)
    pos_sum = 0.0
    for c in range(NCORE):
        # partial columns are rotated chunk indices: u -> chunk (u + c) % 64
        part = res.results[c]["partial"].astype(np.float64)
        total += np.roll(part, c, axis=1)
        pos_sum += float(res.results[c]["pos"].astype(np.float64).sum())
    rowsum = total.T.reshape(N)
    lse = np.log(rowsum - E2)
    loss = (lse.sum() - pos_sum) / N
    return np.asarray(loss, dtype=np.float32)
